# revision 14
# baseline (speedup 1.0000x reference)
"""Multi-head self-attention (B=2, T=2048, D=1024, H=16, causal, interleaved RoPE)
for 8 NeuronCores.

Sharding: core c handles batch b = c//4 and head group g = c%4 (heads 4g..4g+3).
Data parallel over B, tensor parallel over heads; each core emits a partial
x @ Wo.T (row-parallel) and the host sums the 4 partials per batch.

Per-core layout (all device math in bf16 matmuls with fp32 PSUM accumulation):
  - x[b].T is staged as [1024, 2048] so every matmul runs with contraction on
    the partition dim and no on-device transposes anywhere.
  - Q/K are produced transposed ([dh, t], two heads stacked per 128 partitions)
    which makes QK^T emit S^T tiles [128 keys, q] directly, and lets the AV
    matmul consume exp(S^T) as the moving operand with V [t, dh] stationary.
  - Interleaved RoPE (x0,x1,..) -> (-x1,x0,..) is a fixed +-1 pair-permutation:
    applied as one 128x128 matmul (block-diag for the 2 stacked heads) plus
    cos/sin elementwise combines on VectorE.
  - Causality: tiles fully above the diagonal are skipped; diagonal tiles are
    zeroed after exp with precomputed 0/1 bf16 masks.
  - Softmax denominators: V is augmented with a ones column (M=65 matmul), so
    row 64 of each AV PSUM tile accumulates sum(exp(s)); normalization is
    reciprocal(broadcast(sums)) on GpSimd/VectorE, folded into the PSUM->SBUF
    copy of the AV result.
"""

import math
import os
from contextlib import ExitStack

import ml_dtypes
import numpy as np

B, T, D, H = 2, 2048, 1024, 16
DH = D // H          # 64
HPC = H // 4         # 4 heads per core
FPC = HPC * DH       # 256 features per core

bf16 = ml_dtypes.bfloat16

_CACHE: dict = {}


def _build_program(debug_taps=False):
    import concourse.bacc as bacc
    import concourse.mybir as mybir
    import concourse.tile as tile

    nc = bacc.Bacc("TRN2", target_bir_lowering=False, debug=False, num_devices=8)
    dt = mybir.dt
    AF = mybir.ActivationFunctionType

    dbg = {}
    if debug_taps:
        dbg["qf"] = nc.dram_tensor("dbg_qf", [4, 128, T], dt.bfloat16,
                                   kind="ExternalOutput").ap()
        dbg["v"] = nc.dram_tensor("dbg_v", [16, 128, 260], dt.bfloat16,
                                  kind="ExternalOutput").ap()
        dbg["avp"] = nc.dram_tensor("dbg_avp", [2, 128, T], dt.bfloat16,
                                    kind="ExternalOutput").ap()
        dbg["sums"] = nc.dram_tensor("dbg_sums", [4, 2, 2, 512], dt.float32,
                                     kind="ExternalOutput").ap()
        dbg["bc"] = nc.dram_tensor("dbg_bc", [4, 2, 2, 64, 512], dt.float32,
                                   kind="ExternalOutput").ap()
        dbg["rcp"] = nc.dram_tensor("dbg_rcp", [4, 2, 2, 64, 512], dt.float32,
                                    kind="ExternalOutput").ap()

    xt_d = nc.dram_tensor("xt", [D, T], dt.bfloat16, kind="ExternalInput").ap()
    wq_d = nc.dram_tensor("wq", [D, FPC], dt.bfloat16, kind="ExternalInput").ap()
    wk_d = nc.dram_tensor("wk", [D, FPC], dt.bfloat16, kind="ExternalInput").ap()
    wv_d = nc.dram_tensor("wv", [D, FPC], dt.bfloat16, kind="ExternalInput").ap()
    wo_d = nc.dram_tensor("wo", [2, 128, D], dt.bfloat16, kind="ExternalInput").ap()
    cos_d = nc.dram_tensor("cost", [128, T], dt.float32, kind="ExternalInput").ap()
    sin_d = nc.dram_tensor("sint", [128, T], dt.float32, kind="ExternalInput").ap()
    rot_d = nc.dram_tensor("rotm", [128, 128], dt.bfloat16, kind="ExternalInput").ap()
    msk_d = nc.dram_tensor("masks", [128, 2048], dt.bfloat16, kind="ExternalInput").ap()
    out_d = nc.dram_tensor("out", [T, D], dt.float32, kind="ExternalOutput").ap()

    NI = D // 128    # 8 contraction chunks for projections
    NT = T // 128    # 16 token tiles
    SCALE = 1.0 / math.sqrt(DH)

    with tile.TileContext(nc) as tc, ExitStack() as ctx:
        consts = ctx.enter_context(tc.tile_pool(name="consts", bufs=1))
        xt_pool = ctx.enter_context(tc.tile_pool(name="xt", bufs=NI))
        w_pool = ctx.enter_context(tc.tile_pool(name="w", bufs=NI))
        qk_pool = ctx.enter_context(tc.tile_pool(name="qk", bufs=1))
        v_pool = ctx.enter_context(tc.tile_pool(name="v", bufs=NT))
        av_sb_pool = ctx.enter_context(tc.tile_pool(name="avsb", bufs=1))
        work = ctx.enter_context(tc.tile_pool(name="work", bufs=2))
        qb_pool = ctx.enter_context(tc.tile_pool(name="qb", bufs=2))
        es_pool = ctx.enter_context(tc.tile_pool(name="es", bufs=4))
        nrm_pool = ctx.enter_context(tc.tile_pool(name="nrm", bufs=2))
        out_pool = ctx.enter_context(tc.tile_pool(name="osb", bufs=2))

        # ---- loads ----
        cost = consts.tile([128, T], dt.float32, tag="cost", name="cost")
        nc.sync.dma_start(cost[:], cos_d)
        sint = consts.tile([128, T], dt.float32, tag="sint", name="sint")
        nc.sync.dma_start(sint[:], sin_d)
        rotm = consts.tile([128, 128], dt.bfloat16, tag="rotm", name="rotm")
        nc.sync.dma_start(rotm[:], rot_d)
        masks = consts.tile([128, 2048], dt.bfloat16, tag="masks", name="masks")
        nc.sync.dma_start(masks[:], msk_d)
        wo_sb = [consts.tile([128, D], dt.bfloat16, tag=f"wo{p}", name=f"wo{p}") for p in range(2)]
        for p in range(2):
            nc.sync.dma_start(wo_sb[p][:], wo_d[p])
        ones = consts.tile([DH + 1, DH], dt.float32, tag="ones", name="ones")
        nc.vector.memset(ones[:], 1.0)

        xt_sb = []
        for i in range(NI):
            t = xt_pool.tile([128, T], dt.bfloat16, tag="xt", name="xt")
            nc.sync.dma_start(t[:], xt_d[128 * i : 128 * (i + 1), :])
            xt_sb.append(t)
        wq_sb, wk_sb, wv_sb = [], [], []
        for name, dram, lst in (("wq", wq_d, wq_sb), ("wk", wk_d, wk_sb),
                                ("wv", wv_d, wv_sb)):
            for i in range(NI):
                t = w_pool.tile([128, FPC], dt.bfloat16, tag=name)
                nc.sync.dma_start(t[:], dram[128 * i : 128 * (i + 1), :])
                lst.append(t)

        # ---- phase B: Q^T / K^T projections + RoPE ----
        # qf[0], qf[1]: roped Q^T pairs (heads 01, 23); qf[2], qf[3]: roped K^T.
        qf = []
        phase_b = ExitStack()
        ps_qk = phase_b.enter_context(tc.tile_pool(name="psqk", bufs=2, space="PSUM"))
        ps_rot = phase_b.enter_context(tc.tile_pool(name="psrot", bufs=1, space="PSUM"))
        for ft in range(4):
            dst = qk_pool.tile([128, T], dt.bfloat16, tag=f"qk{ft}", name=f"qk{ft}")
            qf.append(dst)
            wsb = wq_sb if ft < 2 else wk_sb
            col0 = 128 * (ft % 2)
            for half in range(2):
                h0 = 1024 * half
                psq = ps_qk.tile([128, 1024], dt.float32, tag="psqk", name="psqk")
                for i in range(NI):
                    for j in range(2):
                        nc.tensor.matmul(
                            psq[:, 512 * j : 512 * (j + 1)],
                            wsb[i][:, col0 : col0 + 128],
                            xt_sb[i][:, h0 + 512 * j : h0 + 512 * (j + 1)],
                            start=(i == 0), stop=(i == NI - 1),
                        )
                qb = qb_pool.tile([128, 1024], dt.bfloat16, tag="qb", name="qb")
                nc.scalar.copy(qb[:], psq[:])
                psr = ps_rot.tile([128, 1024], dt.float32, tag="psrot", name="psrot")
                for j in range(2):
                    nc.tensor.matmul(
                        psr[:, 512 * j : 512 * (j + 1)],
                        rotm[:],
                        qb[:, 512 * j : 512 * (j + 1)],
                        start=True, stop=True,
                    )
                t1 = work.tile([128, 1024], dt.float32, tag="t1", name="t1")
                nc.vector.tensor_mul(t1[:], psq[:], cost[:, h0 : h0 + 1024])
                t2 = work.tile([128, 1024], dt.float32, tag="t2", name="t2")
                nc.vector.tensor_mul(t2[:], psr[:], sint[:, h0 : h0 + 1024])
                nc.vector.tensor_add(dst[:, h0 : h0 + 1024], t1[:], t2[:])
            if debug_taps:
                nc.sync.dma_start(dbg["qf"][ft], dst[:])

        phase_b.close()

        # ---- phase C: V (natural layout, ones-augmented) ----
        # vsb[tt]: [128 tok, 4*65]; head h -> cols 65h..65h+63, ones at 65h+64.
        vsb = []
        phase_c = ExitStack()
        ps_v = phase_c.enter_context(tc.tile_pool(name="psv", bufs=2, space="PSUM"))
        for tt in range(NT):
            pv = ps_v.tile([128, FPC], dt.float32, tag="psv", name="psv")
            for i in range(NI):
                nc.tensor.matmul(
                    pv[:],
                    xt_sb[i][:, 128 * tt : 128 * (tt + 1)],
                    wv_sb[i][:],
                    start=(i == 0), stop=(i == NI - 1),
                )
            vt = v_pool.tile([128, HPC * (DH + 1)], dt.bfloat16, tag="v", name="v")
            vsb.append(vt)
            v3 = vt[:].rearrange("p (h e) -> p h e", e=DH + 1)
            nc.vector.memset(v3[:, :, DH : DH + 1], 1.0)
            nc.vector.tensor_copy(
                v3[:, :, 0:DH], pv[:].rearrange("p (h e) -> p h e", e=DH)
            )
            if debug_taps:
                nc.sync.dma_start(dbg["v"][tt], vt[:])

        # ---- phase D: attention per head ----
        # av_pair[p]: [128, T] bf16; rows 0-63 head 2p, rows 64-127 head 2p+1.
        phase_c.close()
        av_pair = [av_sb_pool.tile([128, T], dt.bfloat16, tag=f"avp{p}", name=f"avp{p}")
                   for p in range(2)]
        phase_d = ExitStack()
        ps_s = phase_d.enter_context(tc.tile_pool(name="pss", bufs=2, space="PSUM"))
        ps_av = phase_d.enter_context(tc.tile_pool(name="psav", bufs=3, space="PSUM"))
        ps_bc = phase_d.enter_context(tc.tile_pool(name="psbc", bufs=1, space="PSUM"))
        for h in range(HPC):
            pair, base = h // 2, 64 * (h % 2)
            kt_ap = qf[2 + pair]
            qt_ap = qf[pair]
            for qs in range(2):
                q0 = 1024 * qs
                nkt = 8 * qs + 8
                av = [ps_av.tile([DH + 1, 512], dt.float32, tag="psav", name="psav")
                      for _ in range(2)]
                for kt in range(nkt):
                    full = kt < 8 * qs + 4
                    c0 = q0 if full else q0 + 512
                    L = q0 + 1024 - c0
                    pss = ps_s.tile([128, L], dt.float32, tag="pss", name="pss")
                    lhsT = kt_ap[base : base + 64, 128 * kt : 128 * (kt + 1)]
                    for j in range(L // 512):
                        nc.tensor.matmul(
                            pss[:, 512 * j : 512 * (j + 1)],
                            lhsT,
                            qt_ap[base : base + 64,
                                  c0 + 512 * j : c0 + 512 * (j + 1)],
                            start=True, stop=True,
                        )
                    es = es_pool.tile([128, L], dt.bfloat16, tag="es", name="es")
                    nc.scalar.activation(es[:], pss[:], AF.Exp, scale=SCALE)
                    if kt >= 8 * qs + 4:
                        r = kt - (8 * qs + 4)
                        nc.vector.tensor_mul(
                            es[:], es[:], masks[:, 512 * r : 512 * (r + 1)])
                    elif kt >= 8 * qs:
                        r = kt - 8 * qs
                        nc.vector.tensor_mul(
                            es[:, 0:512], es[:, 0:512],
                            masks[:, 512 * r : 512 * (r + 1)])
                    vh = vsb[kt][:, (DH + 1) * h : (DH + 1) * (h + 1)]
                    if full:
                        nc.tensor.matmul(av[0][:], vh, es[:, 0:512],
                                         start=(kt == 0), stop=(kt == 8 * qs + 3))
                        nc.tensor.matmul(av[1][:], vh, es[:, 512:1024],
                                         start=(kt == 0), stop=(kt == nkt - 1))
                    else:
                        nc.tensor.matmul(av[1][:], vh, es[:, 0:512],
                                         start=False, stop=(kt == nkt - 1))
                for j in range(2):
                    qc = q0 + 512 * j
                    sums = nrm_pool.tile([DH + 1, 512], dt.float32, tag="sums", name="sums")
                    nc.vector.tensor_copy(sums[DH : DH + 1, :],
                                          av[j][DH : DH + 1, :])
                    bc = ps_bc.tile([DH, 512], dt.float32, tag="psbc", name="psbc")
                    nc.tensor.matmul(bc[:], ones[DH : DH + 1, :],
                                     sums[DH : DH + 1, :], start=True, stop=True)
                    rcp = nrm_pool.tile([DH, 512], dt.float32, tag="rcp", name="rcp")
                    nc.vector.reciprocal(rcp[:], bc[:])
                    if debug_taps:
                        nc.sync.dma_start(dbg["sums"][h, qs, j], sums[DH : DH + 1, :])
                        nc.sync.dma_start(dbg["rcp"][h, qs, j], rcp[:])
                    if h % 2 == 0:
                        nc.vector.tensor_mul(
                            av_pair[pair][0:DH, qc : qc + 512],
                            av[j][0:DH, :], rcp[:])
                    else:
                        tmp = nrm_pool.tile([DH, 512], dt.bfloat16, tag="avtmp", name="avtmp")
                        nc.vector.tensor_mul(tmp[:], av[j][0:DH, :], rcp[:])
                        nc.sync.dma_start(
                            av_pair[pair][DH:128, qc : qc + 512], tmp[:])

        phase_d.close()
        if debug_taps:
            for p in range(2):
                nc.sync.dma_start(dbg["avp"][p], av_pair[p][:])

        # ---- phase E: partial Wo product ----
        ps_o = ctx.enter_context(tc.tile_pool(name="pso", bufs=2, space="PSUM"))
        for tt in range(NT):
            po = ps_o.tile([128, D], dt.float32, tag="pso", name="pso")
            for half in range(2):
                for p in range(2):
                    nc.tensor.matmul(
                        po[:, 512 * half : 512 * (half + 1)],
                        av_pair[p][:, 128 * tt : 128 * (tt + 1)],
                        wo_sb[p][:, 512 * half : 512 * (half + 1)],
                        start=(p == 0), stop=(p == 1),
                    )
            ot = out_pool.tile([128, D], dt.float32, tag="osb", name="osb")
            if tt % 2 == 0:
                nc.scalar.copy(ot[:], po[:])
            else:
                nc.vector.tensor_copy(ot[:], po[:])
            nc.sync.dma_start(out_d[128 * tt : 128 * (tt + 1), :], ot[:])

    nc.compile()
    return nc


def _host_inputs(x, Wqkv, Wo):
    pos = np.arange(T, dtype=np.float32)
    freqs = np.exp(-math.log(10000.0) * np.arange(0, DH, 2, dtype=np.float32) / DH)
    fi = np.repeat(freqs, 2)                      # freq for dims 0..63
    ang = pos[None, :] * fi[:, None]              # [64, T]
    cost = np.concatenate([np.cos(ang)] * 2, 0).astype(np.float32)   # [128, T]
    sint = np.concatenate([np.sin(ang)] * 2, 0).astype(np.float32)

    P = np.zeros((DH, DH), np.float32)
    for i in range(DH // 2):
        P[2 * i, 2 * i + 1] = -1.0
        P[2 * i + 1, 2 * i] = 1.0
    P2 = np.zeros((128, 128), np.float32)
    P2[:DH, :DH] = P
    P2[DH:, DH:] = P
    rotm = np.ascontiguousarray(P2.T).astype(bf16)

    masks = np.zeros((128, 2048), np.float32)
    kk = np.arange(128)[:, None]
    qq = np.arange(512)[None, :]
    for r in range(4):
        masks[:, 512 * r : 512 * (r + 1)] = (kk + 128 * r <= qq)
    masks = masks.astype(bf16)

    in_maps = []
    for c in range(8):
        b, g = divmod(c, 4)
        r0 = FPC * g
        in_maps.append({
            "xt": np.ascontiguousarray(x[b].T).astype(bf16),
            "wq": np.ascontiguousarray(Wqkv[r0 : r0 + FPC, :].T).astype(bf16),
            "wk": np.ascontiguousarray(Wqkv[D + r0 : D + r0 + FPC, :].T).astype(bf16),
            "wv": np.ascontiguousarray(
                Wqkv[2 * D + r0 : 2 * D + r0 + FPC, :].T).astype(bf16),
            "wo": np.ascontiguousarray(
                Wo[:, r0 : r0 + FPC].T.reshape(2, 128, D)).astype(bf16),
            "cost": cost, "sint": sint, "rotm": rotm, "masks": masks,
        })
    return in_maps


def kernel(x, Wqkv, Wo):
    from concourse.bass_utils import run_bass_kernel_spmd

    if "nc" not in _CACHE:
        _CACHE["nc"] = _build_program()
    nc = _CACHE["nc"]

    in_maps = _host_inputs(np.asarray(x), np.asarray(Wqkv), np.asarray(Wo))
    trace = os.environ.get("KERNEL_TRACE") == "1"
    res = run_bass_kernel_spmd(nc, in_maps, core_ids=list(range(8)), trace=trace)
    if trace and res.exec_time_ns is not None:
        print(f"HW exec time: {res.exec_time_ns} ns")

    out = np.zeros((B, T, D), np.float32)
    for c in range(8):
        out[c // 4] += res.results[c]["out"]
    return out


# revision 16
# speedup vs baseline: 1.0205x; 1.0205x over previous
"""Multi-head self-attention (B=2, T=2048, D=1024, H=16, causal, interleaved RoPE)
for 8 NeuronCores.

Sharding: core c handles batch b = c//4 and head group g = c%4 (heads 4g..4g+3).
Data parallel over B, tensor parallel over heads; each core emits a partial
x @ Wo.T (row-parallel) and the host sums the 4 partials per batch.

Per-core layout (all device math in bf16 matmuls with fp32 PSUM accumulation):
  - x[b].T is staged as [1024, 2048] so every matmul runs with contraction on
    the partition dim and no on-device transposes anywhere.
  - Q/K are produced transposed ([dh, t], two heads stacked per 128 partitions)
    which makes QK^T emit S^T tiles [128 keys, q] directly, and lets the AV
    matmul consume exp(S^T) as the moving operand with V [t, dh] stationary.
  - Interleaved RoPE (x0,x1,..) -> (-x1,x0,..) is a fixed +-1 pair-permutation:
    applied as one 128x128 matmul (block-diag for the 2 stacked heads) plus
    cos/sin elementwise combines on VectorE.
  - Causality: tiles fully above the diagonal are skipped; diagonal tiles are
    zeroed after exp with precomputed 0/1 bf16 masks.
  - Softmax denominators: V is augmented with a ones column (M=65 matmul), so
    row 64 of each AV PSUM tile accumulates sum(exp(s)); normalization is
    reciprocal(broadcast(sums)) on GpSimd/VectorE, folded into the PSUM->SBUF
    copy of the AV result.
"""

import math
import os
from contextlib import ExitStack

import ml_dtypes
import numpy as np

B, T, D, H = 2, 2048, 1024, 16
DH = D // H          # 64
HPC = H // 4         # 4 heads per core
FPC = HPC * DH       # 256 features per core

bf16 = ml_dtypes.bfloat16

_CACHE: dict = {}


def _build_program(debug_taps=False):
    import concourse.bacc as bacc
    import concourse.mybir as mybir
    import concourse.tile as tile

    nc = bacc.Bacc("TRN2", target_bir_lowering=False, debug=False, num_devices=8)
    dt = mybir.dt
    AF = mybir.ActivationFunctionType

    dbg = {}
    if debug_taps:
        dbg["qf"] = nc.dram_tensor("dbg_qf", [4, 128, T], dt.bfloat16,
                                   kind="ExternalOutput").ap()
        dbg["v"] = nc.dram_tensor("dbg_v", [16, 128, 260], dt.bfloat16,
                                  kind="ExternalOutput").ap()
        dbg["avp"] = nc.dram_tensor("dbg_avp", [2, 128, T], dt.bfloat16,
                                    kind="ExternalOutput").ap()
        dbg["sums"] = nc.dram_tensor("dbg_sums", [4, 2, 2, 512], dt.float32,
                                     kind="ExternalOutput").ap()
        dbg["bc"] = nc.dram_tensor("dbg_bc", [4, 2, 2, 64, 512], dt.float32,
                                   kind="ExternalOutput").ap()
        dbg["rcp"] = nc.dram_tensor("dbg_rcp", [4, 2, 2, 64, 512], dt.float32,
                                    kind="ExternalOutput").ap()

    xt_d = nc.dram_tensor("xt", [D, T], dt.bfloat16, kind="ExternalInput").ap()
    wq_d = nc.dram_tensor("wq", [D, FPC], dt.bfloat16, kind="ExternalInput").ap()
    wk_d = nc.dram_tensor("wk", [D, FPC], dt.bfloat16, kind="ExternalInput").ap()
    wv_d = nc.dram_tensor("wv", [D, FPC], dt.bfloat16, kind="ExternalInput").ap()
    wo_d = nc.dram_tensor("wo", [2, 128, D], dt.bfloat16, kind="ExternalInput").ap()
    cos_d = nc.dram_tensor("cost", [128, T], dt.float32, kind="ExternalInput").ap()
    sin_d = nc.dram_tensor("sint", [128, T], dt.float32, kind="ExternalInput").ap()
    rot_d = nc.dram_tensor("rotm", [128, 128], dt.bfloat16, kind="ExternalInput").ap()
    msk_d = nc.dram_tensor("masks", [128, 2048], dt.bfloat16, kind="ExternalInput").ap()
    out_d = nc.dram_tensor("out", [T, D], dt.float32, kind="ExternalOutput").ap()

    NI = D // 128    # 8 contraction chunks for projections
    NT = T // 128    # 16 token tiles
    SCALE = 1.0 / math.sqrt(DH)

    with tile.TileContext(nc) as tc, ExitStack() as ctx:
        consts = ctx.enter_context(tc.tile_pool(name="consts", bufs=1))
        xt_pool = ctx.enter_context(tc.tile_pool(name="xt", bufs=NI))
        w_pool = ctx.enter_context(tc.tile_pool(name="w", bufs=NI))
        qk_pool = ctx.enter_context(tc.tile_pool(name="qk", bufs=1))
        v_pool = ctx.enter_context(tc.tile_pool(name="v", bufs=NT))
        av_sb_pool = ctx.enter_context(tc.tile_pool(name="avsb", bufs=1))
        work = ctx.enter_context(tc.tile_pool(name="work", bufs=2))
        qb_pool = ctx.enter_context(tc.tile_pool(name="qb", bufs=2))
        es_pool = ctx.enter_context(tc.tile_pool(name="es", bufs=6))
        nrm_pool = ctx.enter_context(tc.tile_pool(name="nrm", bufs=2))
        out_pool = ctx.enter_context(tc.tile_pool(name="osb", bufs=2))

        # ---- loads ----
        cost = consts.tile([128, T], dt.float32, tag="cost", name="cost")
        nc.sync.dma_start(cost[:], cos_d)
        sint = consts.tile([128, T], dt.float32, tag="sint", name="sint")
        nc.sync.dma_start(sint[:], sin_d)
        rotm = consts.tile([128, 128], dt.bfloat16, tag="rotm", name="rotm")
        nc.sync.dma_start(rotm[:], rot_d)
        masks = consts.tile([128, 2048], dt.bfloat16, tag="masks", name="masks")
        nc.sync.dma_start(masks[:], msk_d)
        wo_sb = [consts.tile([128, D], dt.bfloat16, tag=f"wo{p}", name=f"wo{p}") for p in range(2)]
        for p in range(2):
            nc.sync.dma_start(wo_sb[p][:], wo_d[p])
        ones = consts.tile([DH + 1, DH], dt.float32, tag="ones", name="ones")
        nc.vector.memset(ones[:], 1.0)

        xt_sb = []
        for i in range(NI):
            t = xt_pool.tile([128, T], dt.bfloat16, tag="xt", name="xt")
            nc.sync.dma_start(t[:], xt_d[128 * i : 128 * (i + 1), :])
            xt_sb.append(t)
        wq_sb, wk_sb, wv_sb = [], [], []
        for name, dram, lst in (("wq", wq_d, wq_sb), ("wk", wk_d, wk_sb),
                                ("wv", wv_d, wv_sb)):
            for i in range(NI):
                t = w_pool.tile([128, FPC], dt.bfloat16, tag=name)
                nc.sync.dma_start(t[:], dram[128 * i : 128 * (i + 1), :])
                lst.append(t)

        # ---- phase B: Q^T / K^T projections + RoPE ----
        # qf[0], qf[1]: roped Q^T pairs (heads 01, 23); qf[2], qf[3]: roped K^T.
        qf = []
        phase_b = ExitStack()
        ps_qk = phase_b.enter_context(tc.tile_pool(name="psqk", bufs=2, space="PSUM"))
        ps_rot = phase_b.enter_context(tc.tile_pool(name="psrot", bufs=1, space="PSUM"))
        for ft in range(4):
            dst = qk_pool.tile([128, T], dt.bfloat16, tag=f"qk{ft}", name=f"qk{ft}")
            qf.append(dst)
            wsb = wq_sb if ft < 2 else wk_sb
            col0 = 128 * (ft % 2)
            for half in range(2):
                h0 = 1024 * half
                psq = ps_qk.tile([128, 1024], dt.float32, tag="psqk", name="psqk")
                for i in range(NI):
                    for j in range(2):
                        nc.tensor.matmul(
                            psq[:, 512 * j : 512 * (j + 1)],
                            wsb[i][:, col0 : col0 + 128],
                            xt_sb[i][:, h0 + 512 * j : h0 + 512 * (j + 1)],
                            start=(i == 0), stop=(i == NI - 1),
                        )
                qb = qb_pool.tile([128, 1024], dt.bfloat16, tag="qb", name="qb")
                nc.scalar.copy(qb[:], psq[:])
                psr = ps_rot.tile([128, 1024], dt.float32, tag="psrot", name="psrot")
                for j in range(2):
                    nc.tensor.matmul(
                        psr[:, 512 * j : 512 * (j + 1)],
                        rotm[:],
                        qb[:, 512 * j : 512 * (j + 1)],
                        start=True, stop=True,
                    )
                t1 = work.tile([128, 1024], dt.float32, tag="t1", name="t1")
                nc.vector.tensor_mul(t1[:], psq[:], cost[:, h0 : h0 + 1024])
                t2 = work.tile([128, 1024], dt.float32, tag="t2", name="t2")
                nc.vector.tensor_mul(t2[:], psr[:], sint[:, h0 : h0 + 1024])
                nc.vector.tensor_add(dst[:, h0 : h0 + 1024], t1[:], t2[:])
            if debug_taps:
                nc.sync.dma_start(dbg["qf"][ft], dst[:])

        phase_b.close()

        # ---- phase C: V (natural layout, ones-augmented) ----
        # vsb[tt]: [128 tok, 4*65]; head h -> cols 65h..65h+63, ones at 65h+64.
        vsb = []
        phase_c = ExitStack()
        ps_v = phase_c.enter_context(tc.tile_pool(name="psv", bufs=2, space="PSUM"))
        for tt in range(NT):
            pv = ps_v.tile([128, FPC], dt.float32, tag="psv", name="psv")
            for i in range(NI):
                nc.tensor.matmul(
                    pv[:],
                    xt_sb[i][:, 128 * tt : 128 * (tt + 1)],
                    wv_sb[i][:],
                    start=(i == 0), stop=(i == NI - 1),
                )
            vt = v_pool.tile([128, HPC * (DH + 1)], dt.bfloat16, tag="v", name="v")
            vsb.append(vt)
            v3 = vt[:].rearrange("p (h e) -> p h e", e=DH + 1)
            nc.vector.memset(v3[:, :, DH : DH + 1], 1.0)
            nc.vector.tensor_copy(
                v3[:, :, 0:DH], pv[:].rearrange("p (h e) -> p h e", e=DH)
            )
            if debug_taps:
                nc.sync.dma_start(dbg["v"][tt], vt[:])

        # ---- phase D: attention per head ----
        # av_pair[p]: [128, T] bf16; rows 0-63 head 2p, rows 64-127 head 2p+1.
        phase_c.close()
        av_pair = [av_sb_pool.tile([128, T], dt.bfloat16, tag=f"avp{p}", name=f"avp{p}")
                   for p in range(2)]
        phase_d = ExitStack()
        ps_s = phase_d.enter_context(tc.tile_pool(name="pss", bufs=4, space="PSUM"))
        ps_av = phase_d.enter_context(tc.tile_pool(name="psav", bufs=4, space="PSUM"))
        for pair in range(2):
            kt_ap = qf[2 + pair]
            qt_ap = qf[pair]
            for qs in range(2):
                q0 = 1024 * qs
                nkt = 8 * qs + 8
                # avs[(hd, j)]: head hd of the pair, 512-chunk j of the segment
                avs = {(hd, j): ps_av.tile([DH + 1, 512], dt.float32,
                                           tag="psav", name="psav")
                       for hd in range(2) for j in range(2)}
                for kt in range(nkt):
                    kts = slice(128 * kt, 128 * (kt + 1))
                    for j in ([0, 1] if kt < 8 * qs + 4 else [1]):
                        absc = 2 * qs + j          # absolute 512-chunk index
                        qc = 512 * absc
                        pss = []
                        for hd in range(2):
                            b0 = 64 * hd
                            ps = ps_s.tile([128, 512], dt.float32,
                                           tag="pss", name="pss")
                            pss.append(ps)
                            nc.tensor.matmul(
                                ps[:], kt_ap[b0 : b0 + 64, kts],
                                qt_ap[b0 : b0 + 64, qc : qc + 512],
                                start=True, stop=True,
                            )
                        ess = []
                        for hd in range(2):
                            es = es_pool.tile([128, 512], dt.bfloat16,
                                              tag="es", name="es")
                            ess.append(es)
                            nc.scalar.activation(es[:], pss[hd][:], AF.Exp,
                                                 scale=SCALE)
                        if 4 * absc <= kt:
                            r = kt - 4 * absc
                            for hd in range(2):
                                nc.vector.tensor_mul(
                                    ess[hd][:], ess[hd][:],
                                    masks[:, 512 * r : 512 * (r + 1)])
                        first, last = (kt == 0), (kt == 4 * absc + 3)
                        for hd in range(2):
                            h = 2 * pair + hd
                            vh = vsb[kt][:, (DH + 1) * h : (DH + 1) * (h + 1)]
                            nc.tensor.matmul(avs[(hd, j)][:], vh, ess[hd][:],
                                             start=first, stop=last)
                for hd in range(2):
                    h = 2 * pair + hd
                    for j in range(2):
                        av = avs[(hd, j)]
                        qc = q0 + 512 * j
                        sums = nrm_pool.tile([DH + 1, 512], dt.float32,
                                             tag="sums", name="sums")
                        nc.vector.tensor_copy(sums[DH : DH + 1, :],
                                              av[DH : DH + 1, :])
                        bc = ps_s.tile([DH, 512], dt.float32, tag="pss",
                                       name="psbc")
                        nc.tensor.matmul(bc[:], ones[DH : DH + 1, :],
                                         sums[DH : DH + 1, :],
                                         start=True, stop=True)
                        rcp = nrm_pool.tile([DH, 512], dt.float32, tag="rcp",
                                            name="rcp")
                        nc.vector.reciprocal_approx_fast(rcp[:], bc[:])
                        if debug_taps:
                            nc.sync.dma_start(dbg["sums"][h, qs, j],
                                              sums[DH : DH + 1, :])
                            nc.sync.dma_start(dbg["rcp"][h, qs, j], rcp[:])
                        if hd == 0:
                            nc.vector.tensor_mul(
                                av_pair[pair][0:DH, qc : qc + 512],
                                av[0:DH, :], rcp[:])
                        else:
                            tmp = nrm_pool.tile([DH, 512], dt.bfloat16,
                                                tag="avtmp", name="avtmp")
                            nc.vector.tensor_mul(tmp[:], av[0:DH, :], rcp[:])
                            nc.sync.dma_start(
                                av_pair[pair][DH:128, qc : qc + 512], tmp[:])

        phase_d.close()
        if debug_taps:
            for p in range(2):
                nc.sync.dma_start(dbg["avp"][p], av_pair[p][:])

        # ---- phase E: partial Wo product ----
        ps_o = ctx.enter_context(tc.tile_pool(name="pso", bufs=2, space="PSUM"))
        for tt in range(NT):
            po = ps_o.tile([128, D], dt.float32, tag="pso", name="pso")
            for half in range(2):
                for p in range(2):
                    nc.tensor.matmul(
                        po[:, 512 * half : 512 * (half + 1)],
                        av_pair[p][:, 128 * tt : 128 * (tt + 1)],
                        wo_sb[p][:, 512 * half : 512 * (half + 1)],
                        start=(p == 0), stop=(p == 1),
                    )
            ot = out_pool.tile([128, D], dt.float32, tag="osb", name="osb")
            if tt % 2 == 0:
                nc.scalar.copy(ot[:], po[:])
            else:
                nc.vector.tensor_copy(ot[:], po[:])
            nc.sync.dma_start(out_d[128 * tt : 128 * (tt + 1), :], ot[:])

    nc.compile()
    return nc


def _host_inputs(x, Wqkv, Wo):
    pos = np.arange(T, dtype=np.float32)
    freqs = np.exp(-math.log(10000.0) * np.arange(0, DH, 2, dtype=np.float32) / DH)
    fi = np.repeat(freqs, 2)                      # freq for dims 0..63
    ang = pos[None, :] * fi[:, None]              # [64, T]
    cost = np.concatenate([np.cos(ang)] * 2, 0).astype(np.float32)   # [128, T]
    sint = np.concatenate([np.sin(ang)] * 2, 0).astype(np.float32)

    P = np.zeros((DH, DH), np.float32)
    for i in range(DH // 2):
        P[2 * i, 2 * i + 1] = -1.0
        P[2 * i + 1, 2 * i] = 1.0
    P2 = np.zeros((128, 128), np.float32)
    P2[:DH, :DH] = P
    P2[DH:, DH:] = P
    rotm = np.ascontiguousarray(P2.T).astype(bf16)

    masks = np.zeros((128, 2048), np.float32)
    kk = np.arange(128)[:, None]
    qq = np.arange(512)[None, :]
    for r in range(4):
        masks[:, 512 * r : 512 * (r + 1)] = (kk + 128 * r <= qq)
    masks = masks.astype(bf16)

    in_maps = []
    for c in range(8):
        b, g = divmod(c, 4)
        r0 = FPC * g
        in_maps.append({
            "xt": np.ascontiguousarray(x[b].T).astype(bf16),
            "wq": np.ascontiguousarray(Wqkv[r0 : r0 + FPC, :].T).astype(bf16),
            "wk": np.ascontiguousarray(Wqkv[D + r0 : D + r0 + FPC, :].T).astype(bf16),
            "wv": np.ascontiguousarray(
                Wqkv[2 * D + r0 : 2 * D + r0 + FPC, :].T).astype(bf16),
            "wo": np.ascontiguousarray(
                Wo[:, r0 : r0 + FPC].T.reshape(2, 128, D)).astype(bf16),
            "cost": cost, "sint": sint, "rotm": rotm, "masks": masks,
        })
    return in_maps


def kernel(x, Wqkv, Wo):
    from concourse.bass_utils import run_bass_kernel_spmd

    if "nc" not in _CACHE:
        _CACHE["nc"] = _build_program()
    nc = _CACHE["nc"]

    in_maps = _host_inputs(np.asarray(x), np.asarray(Wqkv), np.asarray(Wo))
    trace = os.environ.get("KERNEL_TRACE") == "1"
    res = run_bass_kernel_spmd(nc, in_maps, core_ids=list(range(8)), trace=trace)
    if trace and res.exec_time_ns is not None:
        print(f"HW exec time: {res.exec_time_ns} ns")

    out = np.zeros((B, T, D), np.float32)
    for c in range(8):
        out[c // 4] += res.results[c]["out"]
    return out


# revision 17
# speedup vs baseline: 1.1064x; 1.0842x over previous
"""Multi-head self-attention (B=2, T=2048, D=1024, H=16, causal, interleaved RoPE)
for 8 NeuronCores.

Sharding: core c handles batch b = c//4 and head group g = c%4 (heads 4g..4g+3).
Data parallel over B, tensor parallel over heads; each core emits a partial
x @ Wo.T (row-parallel) and the host sums the 4 partials per batch.

Per-core layout (all device math in bf16 matmuls with fp32 PSUM accumulation):
  - x[b].T is staged as [1024, 2048] so every matmul runs with contraction on
    the partition dim and no on-device transposes anywhere.
  - Q/K are produced transposed ([dh, t], two heads stacked per 128 partitions)
    which makes QK^T emit S^T tiles [128 keys, q] directly, and lets the AV
    matmul consume exp(S^T) as the moving operand with V [t, dh] stationary.
  - Interleaved RoPE (x0,x1,..) -> (-x1,x0,..) is a fixed +-1 pair-permutation:
    applied as one 128x128 matmul (block-diag for the 2 stacked heads) plus
    cos/sin elementwise combines on VectorE.
  - Causality: tiles fully above the diagonal are skipped; diagonal tiles are
    zeroed after exp with precomputed 0/1 bf16 masks.
  - Softmax denominators: V is augmented with a ones column (M=65 matmul), so
    row 64 of each AV PSUM tile accumulates sum(exp(s)); normalization is
    reciprocal(broadcast(sums)) on GpSimd/VectorE, folded into the PSUM->SBUF
    copy of the AV result.
"""

import math
import os
from contextlib import ExitStack

import ml_dtypes
import numpy as np

B, T, D, H = 2, 2048, 1024, 16
DH = D // H          # 64
HPC = H // 4         # 4 heads per core
FPC = HPC * DH       # 256 features per core

bf16 = ml_dtypes.bfloat16

_CACHE: dict = {}


def _build_program(debug_taps=False):
    import concourse.bacc as bacc
    import concourse.mybir as mybir
    import concourse.tile as tile

    nc = bacc.Bacc("TRN2", target_bir_lowering=False, debug=False, num_devices=8)
    dt = mybir.dt
    AF = mybir.ActivationFunctionType

    dbg = {}
    if debug_taps:
        dbg["qf"] = nc.dram_tensor("dbg_qf", [4, 128, T], dt.bfloat16,
                                   kind="ExternalOutput").ap()
        dbg["v"] = nc.dram_tensor("dbg_v", [16, 128, 260], dt.bfloat16,
                                  kind="ExternalOutput").ap()
        dbg["avp"] = nc.dram_tensor("dbg_avp", [2, 128, T], dt.bfloat16,
                                    kind="ExternalOutput").ap()
        dbg["sums"] = nc.dram_tensor("dbg_sums", [4, 2, 2, 512], dt.float32,
                                     kind="ExternalOutput").ap()
        dbg["bc"] = nc.dram_tensor("dbg_bc", [4, 2, 2, 64, 512], dt.float32,
                                   kind="ExternalOutput").ap()
        dbg["rcp"] = nc.dram_tensor("dbg_rcp", [4, 2, 2, 64, 512], dt.float32,
                                    kind="ExternalOutput").ap()

    xt_d = nc.dram_tensor("xt", [D, T], dt.bfloat16, kind="ExternalInput").ap()
    wq_d = nc.dram_tensor("wq", [D, FPC], dt.bfloat16, kind="ExternalInput").ap()
    wk_d = nc.dram_tensor("wk", [D, FPC], dt.bfloat16, kind="ExternalInput").ap()
    wv_d = nc.dram_tensor("wv", [D, FPC], dt.bfloat16, kind="ExternalInput").ap()
    wo_d = nc.dram_tensor("wo", [2, 128, D], dt.bfloat16, kind="ExternalInput").ap()
    cos_d = nc.dram_tensor("cost", [128, T], dt.float32, kind="ExternalInput").ap()
    sin_d = nc.dram_tensor("sint", [128, T], dt.float32, kind="ExternalInput").ap()
    rot_d = nc.dram_tensor("rotm", [128, 128], dt.bfloat16, kind="ExternalInput").ap()
    msk_d = nc.dram_tensor("masks", [128, 2048], dt.bfloat16, kind="ExternalInput").ap()
    out_d = nc.dram_tensor("out", [T, D], dt.float32, kind="ExternalOutput").ap()

    NI = D // 128    # 8 contraction chunks for projections
    NT = T // 128    # 16 token tiles
    SCALE = 1.0 / math.sqrt(DH)

    with tile.TileContext(nc) as tc, ExitStack() as ctx:
        consts = ctx.enter_context(tc.tile_pool(name="consts", bufs=1))
        xt_pool = ctx.enter_context(tc.tile_pool(name="xt", bufs=NI))
        w_pool = ctx.enter_context(tc.tile_pool(name="w", bufs=NI))
        qk_pool = ctx.enter_context(tc.tile_pool(name="qk", bufs=1))
        v_pool = ctx.enter_context(tc.tile_pool(name="v", bufs=NT))
        av_sb_pool = ctx.enter_context(tc.tile_pool(name="avsb", bufs=1))
        work = ctx.enter_context(tc.tile_pool(name="work", bufs=2))
        qb_pool = ctx.enter_context(tc.tile_pool(name="qb", bufs=2))
        es_pool = ctx.enter_context(tc.tile_pool(name="es", bufs=6))
        nrm_pool = ctx.enter_context(tc.tile_pool(name="nrm", bufs=2))
        out_pool = ctx.enter_context(tc.tile_pool(name="osb", bufs=2))

        # Static PSUM: 4 x [128, 1024] f32 tensors = all 8 banks, shared by
        # every phase via fixed views so no phase-boundary allocation
        # barriers ever idle the PE (HAM stays warm).
        P = [nc.alloc_psum_tensor(f"P{i}", [128, 1024], dt.float32).ap()
             for i in range(4)]
        pss_slots = [P[0][:, 0:512], P[0][:, 512:1024],
                     P[1][:, 0:512], P[1][:, 512:1024]]
        pss_ctr = [0]
        def next_pss():
            s = pss_slots[pss_ctr[0] % 4]
            pss_ctr[0] += 1
            return s

        # ---- loads ----
        cost = consts.tile([128, T], dt.float32, tag="cost", name="cost")
        nc.sync.dma_start(cost[:], cos_d)
        sint = consts.tile([128, T], dt.float32, tag="sint", name="sint")
        nc.sync.dma_start(sint[:], sin_d)
        rotm = consts.tile([128, 128], dt.bfloat16, tag="rotm", name="rotm")
        nc.sync.dma_start(rotm[:], rot_d)
        masks = consts.tile([128, 2048], dt.bfloat16, tag="masks", name="masks")
        nc.sync.dma_start(masks[:], msk_d)
        wo_sb = [consts.tile([128, D], dt.bfloat16, tag=f"wo{p}", name=f"wo{p}") for p in range(2)]
        for p in range(2):
            nc.sync.dma_start(wo_sb[p][:], wo_d[p])
        ones = consts.tile([DH + 1, DH], dt.float32, tag="ones", name="ones")
        nc.vector.memset(ones[:], 1.0)

        xt_sb = []
        for i in range(NI):
            t = xt_pool.tile([128, T], dt.bfloat16, tag="xt", name="xt")
            nc.sync.dma_start(t[:], xt_d[128 * i : 128 * (i + 1), :])
            xt_sb.append(t)
        wq_sb, wk_sb, wv_sb = [], [], []
        for name, dram, lst in (("wq", wq_d, wq_sb), ("wk", wk_d, wk_sb),
                                ("wv", wv_d, wv_sb)):
            for i in range(NI):
                t = w_pool.tile([128, FPC], dt.bfloat16, tag=name)
                nc.sync.dma_start(t[:], dram[128 * i : 128 * (i + 1), :])
                lst.append(t)

        # ---- phase B: Q^T / K^T projections + RoPE ----
        # qf[0], qf[1]: roped Q^T pairs (heads 01, 23); qf[2], qf[3]: roped K^T.
        qf = []
        for ft in range(4):
            dst = qk_pool.tile([128, T], dt.bfloat16, tag=f"qk{ft}", name=f"qk{ft}")
            qf.append(dst)
            wsb = wq_sb if ft < 2 else wk_sb
            col0 = 128 * (ft % 2)
            for half in range(2):
                h0 = 1024 * half
                psq = P[(2 * ft + half) % 2]
                for i in range(NI):
                    for j in range(2):
                        nc.tensor.matmul(
                            psq[:, 512 * j : 512 * (j + 1)],
                            wsb[i][:, col0 : col0 + 128],
                            xt_sb[i][:, h0 + 512 * j : h0 + 512 * (j + 1)],
                            start=(i == 0), stop=(i == NI - 1),
                        )
                qb = qb_pool.tile([128, 1024], dt.bfloat16, tag="qb", name="qb")
                nc.scalar.copy(qb[:], psq[:])
                psr = P[2]
                for j in range(2):
                    nc.tensor.matmul(
                        psr[:, 512 * j : 512 * (j + 1)],
                        rotm[:],
                        qb[:, 512 * j : 512 * (j + 1)],
                        start=True, stop=True,
                    )
                t1 = work.tile([128, 1024], dt.float32, tag="t1", name="t1")
                nc.vector.tensor_mul(t1[:], psq[:], cost[:, h0 : h0 + 1024])
                t2 = work.tile([128, 1024], dt.float32, tag="t2", name="t2")
                nc.vector.tensor_mul(t2[:], psr[:], sint[:, h0 : h0 + 1024])
                nc.vector.tensor_add(dst[:, h0 : h0 + 1024], t1[:], t2[:])
            if debug_taps:
                nc.sync.dma_start(dbg["qf"][ft], dst[:])

        # ---- phase C: V (natural layout, ones-augmented) ----
        # vsb[tt]: [128 tok, 4*65]; head h -> cols 65h..65h+63, ones at 65h+64.
        vsb = []
        for tt in range(NT):
            pv = P[3][:, 0:FPC] if tt % 2 == 0 else P[3][:, 512 : 512 + FPC]
            for i in range(NI):
                nc.tensor.matmul(
                    pv[:],
                    xt_sb[i][:, 128 * tt : 128 * (tt + 1)],
                    wv_sb[i][:],
                    start=(i == 0), stop=(i == NI - 1),
                )
            vt = v_pool.tile([128, HPC * (DH + 1)], dt.bfloat16, tag="v", name="v")
            vsb.append(vt)
            v3 = vt[:].rearrange("p (h e) -> p h e", e=DH + 1)
            nc.vector.memset(v3[:, :, DH : DH + 1], 1.0)
            nc.vector.tensor_copy(
                v3[:, :, 0:DH], pv[:].rearrange("p (h e) -> p h e", e=DH)
            )
            if debug_taps:
                nc.sync.dma_start(dbg["v"][tt], vt[:])

        # ---- phase D: attention per head ----
        # av_pair[p]: [128, T] bf16; rows 0-63 head 2p, rows 64-127 head 2p+1.
        av_pair = [av_sb_pool.tile([128, T], dt.bfloat16, tag=f"avp{p}", name=f"avp{p}")
                   for p in range(2)]
        av_slots = {(0, 0): P[2][0:DH + 1, 0:512], (0, 1): P[2][0:DH + 1, 512:1024],
                    (1, 0): P[3][0:DH + 1, 0:512], (1, 1): P[3][0:DH + 1, 512:1024]}
        for pair in range(2):
            kt_ap = qf[2 + pair]
            qt_ap = qf[pair]
            for qs in range(2):
                q0 = 1024 * qs
                nkt = 8 * qs + 8
                # avs[(hd, j)]: head hd of the pair, 512-chunk j of the segment
                avs = av_slots
                for kt in range(nkt):
                    kts = slice(128 * kt, 128 * (kt + 1))
                    for j in ([0, 1] if kt < 8 * qs + 4 else [1]):
                        absc = 2 * qs + j          # absolute 512-chunk index
                        qc = 512 * absc
                        pss = []
                        for hd in range(2):
                            b0 = 64 * hd
                            ps = next_pss()
                            pss.append(ps)
                            nc.tensor.matmul(
                                ps[:], kt_ap[b0 : b0 + 64, kts],
                                qt_ap[b0 : b0 + 64, qc : qc + 512],
                                start=True, stop=True,
                            )
                        ess = []
                        for hd in range(2):
                            es = es_pool.tile([128, 512], dt.bfloat16,
                                              tag="es", name="es")
                            ess.append(es)
                            nc.scalar.activation(es[:], pss[hd][:], AF.Exp,
                                                 scale=SCALE)
                        if 4 * absc <= kt:
                            r = kt - 4 * absc
                            mc = 128 * (r + 1)   # cols beyond this are all-ones
                            for hd in range(2):
                                nc.vector.tensor_mul(
                                    ess[hd][:, 0:mc], ess[hd][:, 0:mc],
                                    masks[:, 512 * r : 512 * r + mc])
                        first, last = (kt == 0), (kt == 4 * absc + 3)
                        for hd in range(2):
                            h = 2 * pair + hd
                            vh = vsb[kt][:, (DH + 1) * h : (DH + 1) * (h + 1)]
                            nc.tensor.matmul(avs[(hd, j)][:], vh, ess[hd][:],
                                             start=first, stop=last)
                for hd in range(2):
                    h = 2 * pair + hd
                    for j in range(2):
                        av = avs[(hd, j)]
                        qc = q0 + 512 * j
                        sums = nrm_pool.tile([DH + 1, 512], dt.float32,
                                             tag="sums", name="sums")
                        nc.vector.tensor_copy(sums[DH : DH + 1, :],
                                              av[DH : DH + 1, :])
                        bc = next_pss()[0:DH, :]
                        nc.tensor.matmul(bc[:], ones[DH : DH + 1, :],
                                         sums[DH : DH + 1, :],
                                         start=True, stop=True)
                        rcp = nrm_pool.tile([DH, 512], dt.float32, tag="rcp",
                                            name="rcp")
                        nc.vector.reciprocal_approx_fast(rcp[:], bc[:])
                        if debug_taps:
                            nc.sync.dma_start(dbg["sums"][h, qs, j],
                                              sums[DH : DH + 1, :])
                            nc.sync.dma_start(dbg["rcp"][h, qs, j], rcp[:])
                        if hd == 0:
                            nc.vector.tensor_mul(
                                av_pair[pair][0:DH, qc : qc + 512],
                                av[0:DH, :], rcp[:])
                        else:
                            tmp = nrm_pool.tile([DH, 512], dt.bfloat16,
                                                tag="avtmp", name="avtmp")
                            nc.vector.tensor_mul(tmp[:], av[0:DH, :], rcp[:])
                            nc.sync.dma_start(
                                av_pair[pair][DH:128, qc : qc + 512], tmp[:])

        if debug_taps:
            for p in range(2):
                nc.sync.dma_start(dbg["avp"][p], av_pair[p][:])

        # ---- phase E: partial Wo product ----
        for tt in range(NT):
            po = P[tt % 2]
            for half in range(2):
                for p in range(2):
                    nc.tensor.matmul(
                        po[:, 512 * half : 512 * (half + 1)],
                        av_pair[p][:, 128 * tt : 128 * (tt + 1)],
                        wo_sb[p][:, 512 * half : 512 * (half + 1)],
                        start=(p == 0), stop=(p == 1),
                    )
            ot = out_pool.tile([128, D], dt.float32, tag="osb", name="osb")
            if tt % 2 == 0:
                nc.scalar.copy(ot[:], po[:])
            else:
                nc.vector.tensor_copy(ot[:], po[:])
            nc.sync.dma_start(out_d[128 * tt : 128 * (tt + 1), :], ot[:])

    nc.compile()
    return nc


def _host_inputs(x, Wqkv, Wo):
    pos = np.arange(T, dtype=np.float32)
    freqs = np.exp(-math.log(10000.0) * np.arange(0, DH, 2, dtype=np.float32) / DH)
    fi = np.repeat(freqs, 2)                      # freq for dims 0..63
    ang = pos[None, :] * fi[:, None]              # [64, T]
    cost = np.concatenate([np.cos(ang)] * 2, 0).astype(np.float32)   # [128, T]
    sint = np.concatenate([np.sin(ang)] * 2, 0).astype(np.float32)

    P = np.zeros((DH, DH), np.float32)
    for i in range(DH // 2):
        P[2 * i, 2 * i + 1] = -1.0
        P[2 * i + 1, 2 * i] = 1.0
    P2 = np.zeros((128, 128), np.float32)
    P2[:DH, :DH] = P
    P2[DH:, DH:] = P
    rotm = np.ascontiguousarray(P2.T).astype(bf16)

    masks = np.zeros((128, 2048), np.float32)
    kk = np.arange(128)[:, None]
    qq = np.arange(512)[None, :]
    for r in range(4):
        masks[:, 512 * r : 512 * (r + 1)] = (kk + 128 * r <= qq)
    masks = masks.astype(bf16)

    in_maps = []
    for c in range(8):
        b, g = divmod(c, 4)
        r0 = FPC * g
        in_maps.append({
            "xt": np.ascontiguousarray(x[b].T).astype(bf16),
            "wq": np.ascontiguousarray(Wqkv[r0 : r0 + FPC, :].T).astype(bf16),
            "wk": np.ascontiguousarray(Wqkv[D + r0 : D + r0 + FPC, :].T).astype(bf16),
            "wv": np.ascontiguousarray(
                Wqkv[2 * D + r0 : 2 * D + r0 + FPC, :].T).astype(bf16),
            "wo": np.ascontiguousarray(
                Wo[:, r0 : r0 + FPC].T.reshape(2, 128, D)).astype(bf16),
            "cost": cost, "sint": sint, "rotm": rotm, "masks": masks,
        })
    return in_maps


def kernel(x, Wqkv, Wo):
    from concourse.bass_utils import run_bass_kernel_spmd

    if "nc" not in _CACHE:
        _CACHE["nc"] = _build_program()
    nc = _CACHE["nc"]

    in_maps = _host_inputs(np.asarray(x), np.asarray(Wqkv), np.asarray(Wo))
    trace = os.environ.get("KERNEL_TRACE") == "1"
    res = run_bass_kernel_spmd(nc, in_maps, core_ids=list(range(8)), trace=trace)
    if trace and res.exec_time_ns is not None:
        print(f"HW exec time: {res.exec_time_ns} ns")

    out = np.zeros((B, T, D), np.float32)
    for c in range(8):
        out[c // 4] += res.results[c]["out"]
    return out


# revision 18
# speedup vs baseline: 1.1799x; 1.0664x over previous
"""Multi-head self-attention (B=2, T=2048, D=1024, H=16, causal, interleaved RoPE)
for 8 NeuronCores.

Sharding: core c handles batch b = c//4 and head group g = c%4 (heads 4g..4g+3).
Data parallel over B, tensor parallel over heads; each core emits a partial
x @ Wo.T (row-parallel) and the host sums the 4 partials per batch.

Per-core layout (all device math in bf16 matmuls with fp32 PSUM accumulation):
  - x[b].T is staged as [1024, 2048] so every matmul runs with contraction on
    the partition dim and no on-device transposes anywhere.
  - Q/K are produced transposed ([dh, t], two heads stacked per 128 partitions)
    which makes QK^T emit S^T tiles [128 keys, q] directly, and lets the AV
    matmul consume exp(S^T) as the moving operand with V [t, dh] stationary.
  - Interleaved RoPE (x0,x1,..) -> (-x1,x0,..) is a fixed +-1 pair-permutation:
    applied as one 128x128 matmul (block-diag for the 2 stacked heads) plus
    cos/sin elementwise combines on VectorE.
  - Causality: tiles fully above the diagonal are skipped; diagonal tiles are
    zeroed after exp with precomputed 0/1 bf16 masks.
  - Softmax denominators: V is augmented with a ones column (M=65 matmul), so
    row 64 of each AV PSUM tile accumulates sum(exp(s)); normalization is
    reciprocal(broadcast(sums)) on GpSimd/VectorE, folded into the PSUM->SBUF
    copy of the AV result.
"""

import math
import os
from contextlib import ExitStack

import ml_dtypes
import numpy as np

B, T, D, H = 2, 2048, 1024, 16
DH = D // H          # 64
HPC = H // 4         # 4 heads per core
FPC = HPC * DH       # 256 features per core

bf16 = ml_dtypes.bfloat16

_CACHE: dict = {}


def _build_program(debug_taps=False):
    import concourse.bacc as bacc
    import concourse.mybir as mybir
    import concourse.tile as tile

    nc = bacc.Bacc("TRN2", target_bir_lowering=False, debug=False, num_devices=8)
    dt = mybir.dt
    AF = mybir.ActivationFunctionType

    dbg = {}
    if debug_taps:
        dbg["qf"] = nc.dram_tensor("dbg_qf", [4, 128, T], dt.bfloat16,
                                   kind="ExternalOutput").ap()
        dbg["v"] = nc.dram_tensor("dbg_v", [16, 128, 260], dt.bfloat16,
                                  kind="ExternalOutput").ap()
        dbg["avp"] = nc.dram_tensor("dbg_avp", [2, 128, T], dt.bfloat16,
                                    kind="ExternalOutput").ap()
        dbg["sums"] = nc.dram_tensor("dbg_sums", [4, 2, 2, 512], dt.float32,
                                     kind="ExternalOutput").ap()
        dbg["bc"] = nc.dram_tensor("dbg_bc", [4, 2, 2, 64, 512], dt.float32,
                                   kind="ExternalOutput").ap()
        dbg["rcp"] = nc.dram_tensor("dbg_rcp", [4, 2, 2, 64, 512], dt.float32,
                                    kind="ExternalOutput").ap()

    xt_d = nc.dram_tensor("xt", [D, T], dt.bfloat16, kind="ExternalInput").ap()
    wq_d = nc.dram_tensor("wq", [D, FPC], dt.bfloat16, kind="ExternalInput").ap()
    wk_d = nc.dram_tensor("wk", [D, FPC], dt.bfloat16, kind="ExternalInput").ap()
    wv_d = nc.dram_tensor("wv", [D, FPC], dt.bfloat16, kind="ExternalInput").ap()
    wo_d = nc.dram_tensor("wo", [2, 128, D], dt.bfloat16, kind="ExternalInput").ap()
    cos_d = nc.dram_tensor("cost", [128, T], dt.float32, kind="ExternalInput").ap()
    sin_d = nc.dram_tensor("sint", [128, T], dt.float32, kind="ExternalInput").ap()
    rot_d = nc.dram_tensor("rotm", [128, 128], dt.bfloat16, kind="ExternalInput").ap()
    msk_d = nc.dram_tensor("masks", [128, 2048], dt.bfloat16, kind="ExternalInput").ap()
    out_d = nc.dram_tensor("out", [T, D], dt.float32, kind="ExternalOutput").ap()

    NI = D // 128    # 8 contraction chunks for projections
    NT = T // 128    # 16 token tiles
    SCALE = 1.0 / math.sqrt(DH)

    with tile.TileContext(nc) as tc, ExitStack() as ctx:
        consts = ctx.enter_context(tc.tile_pool(name="consts", bufs=1))
        xt_pool = ctx.enter_context(tc.tile_pool(name="xt", bufs=NI))
        w_pool = ctx.enter_context(tc.tile_pool(name="w", bufs=NI))
        qk_pool = ctx.enter_context(tc.tile_pool(name="qk", bufs=1))
        v_pool = ctx.enter_context(tc.tile_pool(name="v", bufs=NT))
        av_sb_pool = ctx.enter_context(tc.tile_pool(name="avsb", bufs=1))
        work = ctx.enter_context(tc.tile_pool(name="work", bufs=2))
        qb_pool = ctx.enter_context(tc.tile_pool(name="qb", bufs=2))
        es_pool = ctx.enter_context(tc.tile_pool(name="es", bufs=6))
        nrm_pool = ctx.enter_context(tc.tile_pool(name="nrm", bufs=2))
        out_pool = ctx.enter_context(tc.tile_pool(name="osb", bufs=2))

        # Static PSUM: 4 x [128, 1024] f32 tensors = all 8 banks, shared by
        # every phase via fixed views so no phase-boundary allocation
        # barriers ever idle the PE (HAM stays warm).
        P = [nc.alloc_psum_tensor(f"P{i}", [128, 1024], dt.float32).ap()
             for i in range(4)]
        pss_ctr = [0]
        def next_pspair():
            s = P[pss_ctr[0] % 2]
            pss_ctr[0] += 1
            return s

        # ---- loads ----
        cost = consts.tile([128, T], dt.float32, tag="cost", name="cost")
        nc.sync.dma_start(cost[:], cos_d)
        sint = consts.tile([128, T], dt.float32, tag="sint", name="sint")
        nc.sync.dma_start(sint[:], sin_d)
        rotm = consts.tile([128, 128], dt.bfloat16, tag="rotm", name="rotm")
        nc.sync.dma_start(rotm[:], rot_d)
        masks = consts.tile([128, 2048], dt.bfloat16, tag="masks", name="masks")
        nc.sync.dma_start(masks[:], msk_d)
        wo_sb = [consts.tile([128, D], dt.bfloat16, tag=f"wo{p}", name=f"wo{p}") for p in range(2)]
        for p in range(2):
            nc.sync.dma_start(wo_sb[p][:], wo_d[p])
        ones = consts.tile([DH + 1, DH], dt.float32, tag="ones", name="ones")
        nc.vector.memset(ones[:], 1.0)

        xt_sb = []
        for i in range(NI):
            t = xt_pool.tile([128, T], dt.bfloat16, tag="xt", name="xt")
            nc.sync.dma_start(t[:], xt_d[128 * i : 128 * (i + 1), :])
            xt_sb.append(t)
        wq_sb, wk_sb, wv_sb = [], [], []
        for name, dram, lst in (("wq", wq_d, wq_sb), ("wk", wk_d, wk_sb),
                                ("wv", wv_d, wv_sb)):
            for i in range(NI):
                t = w_pool.tile([128, FPC], dt.bfloat16, tag=name)
                nc.sync.dma_start(t[:], dram[128 * i : 128 * (i + 1), :])
                lst.append(t)

        # ---- phase B: Q^T / K^T projections + RoPE ----
        # qf[0], qf[1]: roped Q^T pairs (heads 01, 23); qf[2], qf[3]: roped K^T.
        qf = []
        for ft in range(4):
            dst = qk_pool.tile([128, T], dt.bfloat16, tag=f"qk{ft}", name=f"qk{ft}")
            qf.append(dst)
            wsb = wq_sb if ft < 2 else wk_sb
            col0 = 128 * (ft % 2)
            for half in range(2):
                h0 = 1024 * half
                psq = P[(2 * ft + half) % 2]
                for i in range(NI):
                    for j in range(2):
                        nc.tensor.matmul(
                            psq[:, 512 * j : 512 * (j + 1)],
                            wsb[i][:, col0 : col0 + 128],
                            xt_sb[i][:, h0 + 512 * j : h0 + 512 * (j + 1)],
                            start=(i == 0), stop=(i == NI - 1),
                        )
                qb = qb_pool.tile([128, 1024], dt.bfloat16, tag="qb", name="qb")
                nc.scalar.copy(qb[:], psq[:])
                psr = P[2]
                for j in range(2):
                    nc.tensor.matmul(
                        psr[:, 512 * j : 512 * (j + 1)],
                        rotm[:],
                        qb[:, 512 * j : 512 * (j + 1)],
                        start=True, stop=True,
                    )
                t1 = work.tile([128, 1024], dt.float32, tag="t1", name="t1")
                nc.vector.tensor_mul(t1[:], psq[:], cost[:, h0 : h0 + 1024])
                t2 = work.tile([128, 1024], dt.float32, tag="t2", name="t2")
                nc.vector.tensor_mul(t2[:], psr[:], sint[:, h0 : h0 + 1024])
                nc.vector.tensor_add(dst[:, h0 : h0 + 1024], t1[:], t2[:])
            if debug_taps:
                nc.sync.dma_start(dbg["qf"][ft], dst[:])

        # ---- phase C: V (natural layout, ones-augmented) ----
        # vsb[tt]: [128 tok, 4*65]; head h -> cols 65h..65h+63, ones at 65h+64.
        vsb = []
        for tt in range(NT):
            pv = P[3][:, 0:FPC] if tt % 2 == 0 else P[3][:, 512 : 512 + FPC]
            for i in range(NI):
                nc.tensor.matmul(
                    pv[:],
                    xt_sb[i][:, 128 * tt : 128 * (tt + 1)],
                    wv_sb[i][:],
                    start=(i == 0), stop=(i == NI - 1),
                )
            vt = v_pool.tile([128, HPC * (DH + 1)], dt.bfloat16, tag="v", name="v")
            vsb.append(vt)
            v3 = vt[:].rearrange("p (h e) -> p h e", e=DH + 1)
            nc.vector.memset(v3[:, :, DH : DH + 1], 1.0)
            nc.scalar.copy(
                v3[:, :, 0:DH], pv[:].rearrange("p (h e) -> p h e", e=DH)
            )
            if debug_taps:
                nc.sync.dma_start(dbg["v"][tt], vt[:])

        # ---- phase D: attention per head ----
        # av_pair[p]: [128, T] bf16; rows 0-63 head 2p, rows 64-127 head 2p+1.
        av_pair = [av_sb_pool.tile([128, T], dt.bfloat16, tag=f"avp{p}", name=f"avp{p}")
                   for p in range(2)]
        av_slots = {(0, 0): P[2][0:DH + 1, 0:512], (0, 1): P[2][0:DH + 1, 512:1024],
                    (1, 0): P[3][0:DH + 1, 0:512], (1, 1): P[3][0:DH + 1, 512:1024]}
        for pair in range(2):
            kt_ap = qf[2 + pair]
            qt_ap = qf[pair]
            for qs in range(2):
                q0 = 1024 * qs
                nkt = 8 * qs + 8
                # avs[(hd, j)]: head hd of the pair, 512-chunk j of the segment
                avs = av_slots
                for kt in range(nkt):
                    kts = slice(128 * kt, 128 * (kt + 1))
                    for j in ([0, 1] if kt < 8 * qs + 4 else [1]):
                        absc = 2 * qs + j          # absolute 512-chunk index
                        qc = 512 * absc
                        psp = next_pspair()
                        for hd in range(2):
                            b0 = 64 * hd
                            nc.tensor.matmul(
                                psp[:, 512 * hd : 512 * (hd + 1)],
                                kt_ap[b0 : b0 + 64, kts],
                                qt_ap[b0 : b0 + 64, qc : qc + 512],
                                start=True, stop=True,
                            )
                        es = es_pool.tile([128, 1024], dt.bfloat16,
                                          tag="es", name="es")
                        nc.scalar.activation(es[:], psp[:], AF.Exp, scale=SCALE)
                        if 4 * absc <= kt:
                            r = kt - 4 * absc
                            mc = 128 * (r + 1)   # cols beyond this are all-ones
                            for hd in range(2):
                                nc.vector.tensor_mul(
                                    es[:, 512 * hd : 512 * hd + mc],
                                    es[:, 512 * hd : 512 * hd + mc],
                                    masks[:, 512 * r : 512 * r + mc])
                        first, last = (kt == 0), (kt == 4 * absc + 3)
                        for hd in range(2):
                            h = 2 * pair + hd
                            vh = vsb[kt][:, (DH + 1) * h : (DH + 1) * (h + 1)]
                            nc.tensor.matmul(avs[(hd, j)][:], vh,
                                             es[:, 512 * hd : 512 * (hd + 1)],
                                             start=first, stop=last)
                for hd in range(2):
                    h = 2 * pair + hd
                    for j in range(2):
                        av = avs[(hd, j)]
                        qc = q0 + 512 * j
                        sums = nrm_pool.tile([DH + 1, 512], dt.float32,
                                             tag="sums", name="sums")
                        nc.vector.tensor_copy(sums[DH : DH + 1, :],
                                              av[DH : DH + 1, :])
                        bc = next_pspair()[0:DH, 0:512]
                        nc.tensor.matmul(bc[:], ones[DH : DH + 1, :],
                                         sums[DH : DH + 1, :],
                                         start=True, stop=True)
                        rcp = nrm_pool.tile([DH, 512], dt.float32, tag="rcp",
                                            name="rcp")
                        nc.vector.reciprocal_approx_fast(rcp[:], bc[:])
                        if debug_taps:
                            nc.sync.dma_start(dbg["sums"][h, qs, j],
                                              sums[DH : DH + 1, :])
                            nc.sync.dma_start(dbg["rcp"][h, qs, j], rcp[:])
                        if hd == 0:
                            nc.vector.tensor_mul(
                                av_pair[pair][0:DH, qc : qc + 512],
                                av[0:DH, :], rcp[:])
                        else:
                            tmp = nrm_pool.tile([DH, 512], dt.bfloat16,
                                                tag="avtmp", name="avtmp")
                            nc.vector.tensor_mul(tmp[:], av[0:DH, :], rcp[:])
                            nc.sync.dma_start(
                                av_pair[pair][DH:128, qc : qc + 512], tmp[:])

        if debug_taps:
            for p in range(2):
                nc.sync.dma_start(dbg["avp"][p], av_pair[p][:])

        # ---- phase E: partial Wo product ----
        for tt in range(NT):
            po = P[tt % 2]
            for half in range(2):
                for p in range(2):
                    nc.tensor.matmul(
                        po[:, 512 * half : 512 * (half + 1)],
                        av_pair[p][:, 128 * tt : 128 * (tt + 1)],
                        wo_sb[p][:, 512 * half : 512 * (half + 1)],
                        start=(p == 0), stop=(p == 1),
                    )
            ot = out_pool.tile([128, D], dt.float32, tag="osb", name="osb")
            if tt % 2 == 0:
                nc.scalar.copy(ot[:], po[:])
            else:
                nc.vector.tensor_copy(ot[:], po[:])
            nc.sync.dma_start(out_d[128 * tt : 128 * (tt + 1), :], ot[:])

    nc.compile()
    return nc


def _host_inputs(x, Wqkv, Wo):
    pos = np.arange(T, dtype=np.float32)
    freqs = np.exp(-math.log(10000.0) * np.arange(0, DH, 2, dtype=np.float32) / DH)
    fi = np.repeat(freqs, 2)                      # freq for dims 0..63
    ang = pos[None, :] * fi[:, None]              # [64, T]
    cost = np.concatenate([np.cos(ang)] * 2, 0).astype(np.float32)   # [128, T]
    sint = np.concatenate([np.sin(ang)] * 2, 0).astype(np.float32)

    P = np.zeros((DH, DH), np.float32)
    for i in range(DH // 2):
        P[2 * i, 2 * i + 1] = -1.0
        P[2 * i + 1, 2 * i] = 1.0
    P2 = np.zeros((128, 128), np.float32)
    P2[:DH, :DH] = P
    P2[DH:, DH:] = P
    rotm = np.ascontiguousarray(P2.T).astype(bf16)

    masks = np.zeros((128, 2048), np.float32)
    kk = np.arange(128)[:, None]
    qq = np.arange(512)[None, :]
    for r in range(4):
        masks[:, 512 * r : 512 * (r + 1)] = (kk + 128 * r <= qq)
    masks = masks.astype(bf16)

    in_maps = []
    for c in range(8):
        b, g = divmod(c, 4)
        r0 = FPC * g
        in_maps.append({
            "xt": np.ascontiguousarray(x[b].T).astype(bf16),
            "wq": np.ascontiguousarray(Wqkv[r0 : r0 + FPC, :].T).astype(bf16),
            "wk": np.ascontiguousarray(Wqkv[D + r0 : D + r0 + FPC, :].T).astype(bf16),
            "wv": np.ascontiguousarray(
                Wqkv[2 * D + r0 : 2 * D + r0 + FPC, :].T).astype(bf16),
            "wo": np.ascontiguousarray(
                Wo[:, r0 : r0 + FPC].T.reshape(2, 128, D)).astype(bf16),
            "cost": cost, "sint": sint, "rotm": rotm, "masks": masks,
        })
    return in_maps


def kernel(x, Wqkv, Wo):
    from concourse.bass_utils import run_bass_kernel_spmd

    if "nc" not in _CACHE:
        _CACHE["nc"] = _build_program()
    nc = _CACHE["nc"]

    in_maps = _host_inputs(np.asarray(x), np.asarray(Wqkv), np.asarray(Wo))
    trace = os.environ.get("KERNEL_TRACE") == "1"
    res = run_bass_kernel_spmd(nc, in_maps, core_ids=list(range(8)), trace=trace)
    if trace and res.exec_time_ns is not None:
        print(f"HW exec time: {res.exec_time_ns} ns")

    out = np.zeros((B, T, D), np.float32)
    for c in range(8):
        out[c // 4] += res.results[c]["out"]
    return out


# revision 21
# speedup vs baseline: 1.3342x; 1.1308x over previous
"""Multi-head self-attention (B=2, T=2048, D=1024, H=16, causal, interleaved RoPE)
for 8 NeuronCores.

Sharding: core c handles batch b = c//4 and head group g = c%4 (heads 4g..4g+3).
Data parallel over B, tensor parallel over heads; each core emits a partial
x @ Wo.T (row-parallel) and the host sums the 4 partials per batch.

Per-core layout (all device math in bf16 matmuls with fp32 PSUM accumulation):
  - x[b].T is staged as [1024, 2048] so every matmul runs with contraction on
    the partition dim and no on-device transposes anywhere.
  - Q/K are produced transposed ([dh, t], two heads stacked per 128 partitions)
    which makes QK^T emit S^T tiles [128 keys, q] directly, and lets the AV
    matmul consume exp(S^T) as the moving operand with V [t, dh] stationary.
  - Interleaved RoPE (x0,x1,..) -> (-x1,x0,..) is a fixed +-1 pair-permutation:
    applied as one 128x128 matmul (block-diag for the 2 stacked heads) plus
    cos/sin elementwise combines on VectorE.
  - Causality: tiles fully above the diagonal are skipped; diagonal tiles are
    zeroed after exp with precomputed 0/1 bf16 masks.
  - Softmax denominators: V is augmented with a ones column (M=65 matmul), so
    row 64 of each AV PSUM tile accumulates sum(exp(s)); normalization is
    reciprocal(broadcast(sums)) on GpSimd/VectorE, folded into the PSUM->SBUF
    copy of the AV result.
"""

import math
import os
from contextlib import ExitStack

import ml_dtypes
import numpy as np

B, T, D, H = 2, 2048, 1024, 16
DH = D // H          # 64
HPC = H // 4         # 4 heads per core
FPC = HPC * DH       # 256 features per core

bf16 = ml_dtypes.bfloat16

_CACHE: dict = {}


def _build_program(debug_taps=False):
    import concourse.bacc as bacc
    import concourse.mybir as mybir
    import concourse.tile as tile

    nc = bacc.Bacc("TRN2", target_bir_lowering=False, debug=False, num_devices=8)
    dt = mybir.dt
    AF = mybir.ActivationFunctionType

    dbg = {}
    if debug_taps:
        dbg["qf"] = nc.dram_tensor("dbg_qf", [4, 128, T], dt.bfloat16,
                                   kind="ExternalOutput").ap()
        dbg["v"] = nc.dram_tensor("dbg_v", [16, 128, 260], dt.bfloat16,
                                  kind="ExternalOutput").ap()
        dbg["avp"] = nc.dram_tensor("dbg_avp", [2, 128, T], dt.bfloat16,
                                    kind="ExternalOutput").ap()
        dbg["sums"] = nc.dram_tensor("dbg_sums", [4, 2, 2, 512], dt.float32,
                                     kind="ExternalOutput").ap()
        dbg["bc"] = nc.dram_tensor("dbg_bc", [4, 2, 2, 64, 512], dt.float32,
                                   kind="ExternalOutput").ap()
        dbg["rcp"] = nc.dram_tensor("dbg_rcp", [4, 2, 2, 64, 512], dt.float32,
                                    kind="ExternalOutput").ap()

    xt_d = nc.dram_tensor("xt", [D, T], dt.bfloat16, kind="ExternalInput").ap()
    wq_d = nc.dram_tensor("wq", [D, FPC], dt.bfloat16, kind="ExternalInput").ap()
    wk_d = nc.dram_tensor("wk", [D, FPC], dt.bfloat16, kind="ExternalInput").ap()
    wv_d = nc.dram_tensor("wv", [D, FPC], dt.bfloat16, kind="ExternalInput").ap()
    wo_d = nc.dram_tensor("wo", [2, 128, D], dt.bfloat16, kind="ExternalInput").ap()
    cos_d = nc.dram_tensor("cost", [128, T], dt.float32, kind="ExternalInput").ap()
    sin_d = nc.dram_tensor("sint", [128, T], dt.float32, kind="ExternalInput").ap()
    rot_d = nc.dram_tensor("rotm", [128, 128], dt.bfloat16, kind="ExternalInput").ap()
    msk_d = nc.dram_tensor("masks", [128, 2048], dt.bfloat16, kind="ExternalInput").ap()
    ones_d = nc.dram_tensor("ones_in", [DH + 1, DH], dt.float32r,
                            kind="ExternalInput").ap()
    out_d = nc.dram_tensor("out", [T, D], dt.float32, kind="ExternalOutput").ap()

    NI = D // 128    # 8 contraction chunks for projections
    NT = T // 128    # 16 token tiles
    SCALE = 1.0 / math.sqrt(DH)

    with tile.TileContext(nc) as tc, ExitStack() as ctx:
        consts = ctx.enter_context(tc.tile_pool(name="consts", bufs=1))
        xt_pool = ctx.enter_context(tc.tile_pool(name="xt", bufs=NI))
        w_pool = ctx.enter_context(tc.tile_pool(name="w", bufs=NI))
        qk_pool = ctx.enter_context(tc.tile_pool(name="qk", bufs=1))
        v_pool = ctx.enter_context(tc.tile_pool(name="v", bufs=NT))
        av_sb_pool = ctx.enter_context(tc.tile_pool(name="avsb", bufs=1))
        work = ctx.enter_context(tc.tile_pool(name="work", bufs=2))
        qb_pool = ctx.enter_context(tc.tile_pool(name="qb", bufs=2))
        es_pool = ctx.enter_context(tc.tile_pool(name="es", bufs=6))
        nrm_pool = ctx.enter_context(tc.tile_pool(name="nrm", bufs=2))
        out_pool = ctx.enter_context(tc.tile_pool(name="osb", bufs=2))

        # Static PSUM: 4 x [128, 1024] f32 tensors = all 8 banks, shared by
        # every phase via fixed views so no phase-boundary allocation
        # barriers ever idle the PE (HAM stays warm).
        P = [nc.alloc_psum_tensor(f"P{i}", [128, 1024], dt.float32).ap()
             for i in range(4)]
        pss_ctr = [0]
        def next_pspair():
            s = P[pss_ctr[0] % 2]
            pss_ctr[0] += 1
            return s

        # ---- loads ----
        cost = consts.tile([128, T], dt.float32, tag="cost", name="cost")
        nc.sync.dma_start(cost[:], cos_d)
        sint = consts.tile([128, T], dt.float32, tag="sint", name="sint")
        nc.sync.dma_start(sint[:], sin_d)
        rotm = consts.tile([128, 128], dt.bfloat16, tag="rotm", name="rotm")
        nc.sync.dma_start(rotm[:], rot_d)
        masks = consts.tile([128, 2048], dt.bfloat16, tag="masks", name="masks")
        nc.sync.dma_start(masks[:], msk_d)
        wo_sb = [consts.tile([128, D], dt.bfloat16, tag=f"wo{p}", name=f"wo{p}") for p in range(2)]
        for p in range(2):
            nc.sync.dma_start(wo_sb[p][:], wo_d[p])
        ones = consts.tile([DH + 1, DH], dt.float32r, tag="ones", name="ones")
        nc.sync.dma_start(ones[:], ones_d)

        xt_sb = []
        for i in range(NI):
            t = xt_pool.tile([128, T], dt.bfloat16, tag="xt", name="xt")
            nc.sync.dma_start(t[:], xt_d[128 * i : 128 * (i + 1), :])
            xt_sb.append(t)
        wq_sb, wk_sb, wv_sb = [], [], []
        for name, dram, lst in (("wq", wq_d, wq_sb), ("wk", wk_d, wk_sb),
                                ("wv", wv_d, wv_sb)):
            for i in range(NI):
                t = w_pool.tile([128, FPC], dt.bfloat16, tag=name)
                nc.sync.dma_start(t[:], dram[128 * i : 128 * (i + 1), :])
                lst.append(t)

        # ---- phase B: Q^T / K^T projections + RoPE ----
        # qf[0], qf[1]: roped Q^T pairs (heads 01, 23); qf[2], qf[3]: roped K^T.
        qf = []
        for ft in range(4):
            dst = qk_pool.tile([128, T], dt.bfloat16, tag=f"qk{ft}", name=f"qk{ft}")
            qf.append(dst)
            wsb = wq_sb if ft < 2 else wk_sb
            col0 = 128 * (ft % 2)
            for half in range(2):
                h0 = 1024 * half
                psq = P[(2 * ft + half) % 2]
                for i in range(NI):
                    for j in range(2):
                        nc.tensor.matmul(
                            psq[:, 512 * j : 512 * (j + 1)],
                            wsb[i][:, col0 : col0 + 128],
                            xt_sb[i][:, h0 + 512 * j : h0 + 512 * (j + 1)],
                            start=(i == 0), stop=(i == NI - 1),
                        )
                qb = qb_pool.tile([128, 1024], dt.bfloat16, tag="qb", name="qb")
                nc.scalar.copy(qb[:], psq[:])
                psr = P[2]
                for j in range(2):
                    nc.tensor.matmul(
                        psr[:, 512 * j : 512 * (j + 1)],
                        rotm[:],
                        qb[:, 512 * j : 512 * (j + 1)],
                        start=True, stop=True,
                    )
                t1 = work.tile([128, 1024], dt.float32, tag="t1", name="t1")
                nc.vector.tensor_mul(t1[:], psq[:], cost[:, h0 : h0 + 1024])
                t2 = work.tile([128, 1024], dt.float32, tag="t2", name="t2")
                nc.vector.tensor_mul(t2[:], psr[:], sint[:, h0 : h0 + 1024])
                nc.gpsimd.tensor_add(dst[:, h0 : h0 + 1024], t1[:], t2[:])
            if debug_taps:
                nc.sync.dma_start(dbg["qf"][ft], dst[:])

        # ---- phase C: V (natural layout, ones-augmented) ----
        # vsb[tt]: [128 tok, 4*65]; head h -> cols 65h..65h+63, ones at 65h+64.
        vsb = []
        for tt in range(NT):
            pv = [P[3][:, 0:FPC], P[3][:, 512 : 512 + FPC],
                  P[2][:, 0:FPC], P[2][:, 512 : 512 + FPC]][tt % 4]
            for i in range(NI):
                nc.tensor.matmul(
                    pv[:],
                    xt_sb[i][:, 128 * tt : 128 * (tt + 1)],
                    wv_sb[i][:],
                    start=(i == 0), stop=(i == NI - 1),
                )
            vt = v_pool.tile([128, HPC * (DH + 1)], dt.bfloat16, tag="v", name="v")
            vsb.append(vt)
            v3 = vt[:].rearrange("p (h e) -> p h e", e=DH + 1)
            nc.vector.memset(v3[:, :, DH : DH + 1], 1.0)
            nc.scalar.copy(
                v3[:, :, 0:DH], pv[:].rearrange("p (h e) -> p h e", e=DH)
            )
            if debug_taps:
                nc.sync.dma_start(dbg["v"][tt], vt[:])

        # ---- phase D: attention per head ----
        # av_pair[p]: [128, T] bf16; rows 0-63 head 2p, rows 64-127 head 2p+1.
        av_pair = [av_sb_pool.tile([128, T], dt.bfloat16, tag=f"avp{p}", name=f"avp{p}")
                   for p in range(2)]
        av_slots = {(0, 0): P[2][0:DH + 1, 0:512], (0, 1): P[2][0:DH + 1, 512:1024],
                    (1, 0): P[3][0:DH + 1, 0:512], (1, 1): P[3][0:DH + 1, 512:1024]}

        def do_norm(pair, qs, hd, j, av):
            h = 2 * pair + hd
            qc = 1024 * qs + 512 * j
            sums = nrm_pool.tile([DH + 1, 512], dt.float32r,
                                 tag="sums", name="sums")
            nc.vector.tensor_copy(sums[DH : DH + 1, :], av[DH : DH + 1, :])
            bc = next_pspair()[0:DH, 0:512]
            nc.tensor.matmul(bc[:], ones[DH : DH + 1, :],
                             sums[DH : DH + 1, :], start=True, stop=True)
            rcp = nrm_pool.tile([DH, 512], dt.float32, tag="rcp", name="rcp")
            nc.vector.reciprocal_approx_fast(rcp[:], bc[:])
            if debug_taps:
                nc.sync.dma_start(dbg["sums"][h, qs, j],
                                  sums[DH : DH + 1, :].bitcast(dt.float32))
                nc.sync.dma_start(dbg["rcp"][h, qs, j], rcp[:])
            if hd == 0:
                nc.vector.tensor_mul(av_pair[pair][0:DH, qc : qc + 512],
                                     av[0:DH, :], rcp[:])
            else:
                tmp = nrm_pool.tile([DH, 512], dt.bfloat16,
                                    tag="avtmp", name="avtmp")
                nc.vector.tensor_mul(tmp[:], av[0:DH, :], rcp[:])
                nc.sync.dma_start(av_pair[pair][DH:128, qc : qc + 512], tmp[:])
        for pair in range(2):
            kt_ap = qf[2 + pair]
            qt_ap = qf[pair]
            for qs in range(2):
                q0 = 1024 * qs
                nkt = 8 * qs + 8
                # avs[(hd, j)]: head hd of the pair, 512-chunk j of the segment
                avs = av_slots
                for kt in range(nkt):
                    kts = slice(128 * kt, 128 * (kt + 1))
                    for j in ([0, 1] if kt < 8 * qs + 4 else [1]):
                        absc = 2 * qs + j          # absolute 512-chunk index
                        qc = 512 * absc
                        psp = next_pspair()
                        for hd in range(2):
                            b0 = 64 * hd
                            nc.tensor.matmul(
                                psp[:, 512 * hd : 512 * (hd + 1)],
                                kt_ap[b0 : b0 + 64, kts],
                                qt_ap[b0 : b0 + 64, qc : qc + 512],
                                start=True, stop=True,
                            )
                        es = es_pool.tile([128, 1024], dt.bfloat16,
                                          tag="es", name="es")
                        nc.scalar.activation(es[:], psp[:], AF.Exp, scale=SCALE)
                        if 4 * absc <= kt:
                            r = kt - 4 * absc
                            mc = 128 * (r + 1)   # cols beyond this are all-ones
                            for hd in range(2):
                                nc.vector.tensor_mul(
                                    es[:, 512 * hd : 512 * hd + mc],
                                    es[:, 512 * hd : 512 * hd + mc],
                                    masks[:, 512 * r : 512 * r + mc])
                        first, last = (kt == 0), (kt == 4 * absc + 3)
                        for hd in range(2):
                            h = 2 * pair + hd
                            vh = vsb[kt][:, (DH + 1) * h : (DH + 1) * (h + 1)]
                            nc.tensor.matmul(avs[(hd, j)][:], vh,
                                             es[:, 512 * hd : 512 * (hd + 1)],
                                             start=first, stop=last)
                    if kt == 8 * qs + 3:
                        for hd in range(2):
                            do_norm(pair, qs, hd, 0, avs[(hd, 0)])
                for hd in range(2):
                    do_norm(pair, qs, hd, 1, avs[(hd, 1)])

        if debug_taps:
            for p in range(2):
                nc.sync.dma_start(dbg["avp"][p], av_pair[p][:])

        # ---- phase E: partial Wo product ----
        for tt in range(NT):
            po = P[tt % 2]
            for half in range(2):
                for p in range(2):
                    nc.tensor.matmul(
                        po[:, 512 * half : 512 * (half + 1)],
                        av_pair[p][:, 128 * tt : 128 * (tt + 1)],
                        wo_sb[p][:, 512 * half : 512 * (half + 1)],
                        start=(p == 0), stop=(p == 1),
                    )
            ot = out_pool.tile([128, D], dt.float32, tag="osb", name="osb")
            if tt % 2 == 0:
                nc.scalar.copy(ot[:], po[:])
            else:
                nc.vector.tensor_copy(ot[:], po[:])
            nc.sync.dma_start(out_d[128 * tt : 128 * (tt + 1), :], ot[:])

    nc.compile()
    return nc


def _host_inputs(x, Wqkv, Wo):
    pos = np.arange(T, dtype=np.float32)
    freqs = np.exp(-math.log(10000.0) * np.arange(0, DH, 2, dtype=np.float32) / DH)
    fi = np.repeat(freqs, 2)                      # freq for dims 0..63
    ang = pos[None, :] * fi[:, None]              # [64, T]
    cost = np.concatenate([np.cos(ang)] * 2, 0).astype(np.float32)   # [128, T]
    sint = np.concatenate([np.sin(ang)] * 2, 0).astype(np.float32)

    P = np.zeros((DH, DH), np.float32)
    for i in range(DH // 2):
        P[2 * i, 2 * i + 1] = -1.0
        P[2 * i + 1, 2 * i] = 1.0
    P2 = np.zeros((128, 128), np.float32)
    P2[:DH, :DH] = P
    P2[DH:, DH:] = P
    rotm = np.ascontiguousarray(P2.T).astype(bf16)

    masks = np.zeros((128, 2048), np.float32)
    kk = np.arange(128)[:, None]
    qq = np.arange(512)[None, :]
    for r in range(4):
        masks[:, 512 * r : 512 * (r + 1)] = (kk + 128 * r <= qq)
    masks = masks.astype(bf16)

    in_maps = []
    for c in range(8):
        b, g = divmod(c, 4)
        r0 = FPC * g
        in_maps.append({
            "xt": np.ascontiguousarray(x[b].T).astype(bf16),
            "wq": np.ascontiguousarray(Wqkv[r0 : r0 + FPC, :].T).astype(bf16),
            "wk": np.ascontiguousarray(Wqkv[D + r0 : D + r0 + FPC, :].T).astype(bf16),
            "wv": np.ascontiguousarray(
                Wqkv[2 * D + r0 : 2 * D + r0 + FPC, :].T).astype(bf16),
            "wo": np.ascontiguousarray(
                Wo[:, r0 : r0 + FPC].T.reshape(2, 128, D)).astype(bf16),
            "cost": cost, "sint": sint, "rotm": rotm, "masks": masks,
            "ones_in": np.ones((DH + 1, DH), np.float32),
        })
    return in_maps


def kernel(x, Wqkv, Wo):
    from concourse.bass_utils import run_bass_kernel_spmd

    if "nc" not in _CACHE:
        _CACHE["nc"] = _build_program()
    nc = _CACHE["nc"]

    in_maps = _host_inputs(np.asarray(x), np.asarray(Wqkv), np.asarray(Wo))
    trace = os.environ.get("KERNEL_TRACE") == "1"
    res = run_bass_kernel_spmd(nc, in_maps, core_ids=list(range(8)), trace=trace)
    if trace and res.exec_time_ns is not None:
        print(f"HW exec time: {res.exec_time_ns} ns")

    out = np.zeros((B, T, D), np.float32)
    for c in range(8):
        out[c // 4] += res.results[c]["out"]
    return out
